# revision 6
# baseline (speedup 1.0000x reference)
"""Trainium2 Bass kernel for CohomologicalAttention.

The reference module is, mathematically, standard multi-head attention:
its "consistency weights" are a softmax over a size-1 axis and are
therefore identically 1.0 (and multiply the scores by 1.0), so the
restriction-map branch contributes nothing to either output.

Sharding (8 NeuronCores): data-parallel over batch (2) x tensor-parallel
over heads (4 heads per core).  Each core computes q/k/v projections for
its 4 heads, the full attention for those heads, and a partial output
projection (row-parallel in Wo); the host sums the 4 partials per batch.

All matmuls run as float32r (TF32-like fast fp32 path, 4x the fp32 rate)
accumulating in fp32 PSUM.  Softmax skips the max-subtraction (scores are
~N(0,1); |scores| < ~7 so exp is safe in fp32) and gets its denominator
for free from an appended ones-column on V.
"""

import numpy as np

B, S, E, H, P = 2, 2048, 1024, 16, 4
DH = E // H          # 64
NH = H // 4          # 4 local heads per core (8 cores = 2 batches x 4 groups)
MLOC = NH * DH       # 256 local q/k/v rows per core
KC = E // 128        # 8 contraction chunks of 128
SW = 512             # matmul moving-dim width (fp32 max)
WIN = 1024           # exp / s window
NWIN = S // WIN      # 2

_CACHE = {}


def _phase_a(nc, mybir, xw, psA, xT, wqT, wkT, wvT,
             bq_sb, bk_sb, bv_sb, ones1, qT_sb, kT_sb, v_sb, F32, F32R):
    wq_sb = xw.tile([128, KC, MLOC], F32R, tag="wq")
    wk_sb = xw.tile([128, KC, MLOC], F32R, tag="wk")
    wv_sb = xw.tile([128, KC, MLOC], F32R, tag="wv")
    nc.sync.dma_start(wq_sb[:], wqT[:].rearrange("(c p) m -> p c m", p=128))
    nc.sync.dma_start(wk_sb[:], wkT[:].rearrange("(c p) m -> p c m", p=128))
    nc.sync.dma_start(wv_sb[:], wvT[:].rearrange("(c p) m -> p c m", p=128))
    xT_sb = xw.tile([128, KC, S], F32R, tag="xT")
    for kb in range(KC):
        nc.sync.dma_start(xT_sb[:, kb, :], xT[kb * 128 : (kb + 1) * 128, :])

    # q/k: psum[d_chunk, s_block] accumulated over e-chunks
    for w_sb, b_sb, dst in ((wq_sb, bq_sb, qT_sb), (wk_sb, bk_sb, kT_sb)):
        for mi in range(2):
            for nb in range(S // SW):
                ps = psA.tile([128, SW], F32, tag="psA")
                for kb in range(KC):
                    nc.tensor.matmul(
                        ps[:],
                        w_sb[:, kb, mi * 128 : (mi + 1) * 128],
                        xT_sb[:, kb, nb * SW : (nb + 1) * SW],
                        start=(kb == 0),
                        stop=(kb == KC - 1),
                    )
                nc.vector.tensor_scalar_add(
                    dst[:, mi, nb * SW : (nb + 1) * SW],
                    ps[:],
                    b_sb[:, mi : mi + 1],
                )

    # v in natural [t, d] layout (+ ones column per head)
    for tb in range(S // 128):
        ps = psA.tile([128, MLOC], F32, tag="psA")
        for kb in range(KC):
            nc.tensor.matmul(
                ps[:],
                xT_sb[:, kb, tb * 128 : (tb + 1) * 128],
                wv_sb[:, kb, :],
                start=(kb == 0),
                stop=(kb == KC - 1),
            )
        nc.vector.tensor_add(
            v_sb[:, tb, :, 0:DH],
            ps[:].rearrange("p (h d) -> p h d", h=NH),
            bv_sb[:].rearrange("p (h d) -> p h d", h=NH),
        )
    nc.vector.tensor_copy(
        v_sb[:, :, :, DH : DH + 1],
        ones1[:, 0:1, None].to_broadcast((128, S // 128, NH, 1)),
    )


def _phase_b(nc, mybir, pp, small, outsb, psS, psAtt,
             qT_sb, kT_sb, v_sb, attT_sb, woT_sb, out, F32, F32R):
    EXP = mybir.ActivationFunctionType.Exp
    for w in range(NWIN):
        for h in range(NH):
            p0 = (h % 2) * 64
            mi = h // 2
            pT_sb = pp.tile([128, S // 128, WIN], F32R, tag="pT")
            for tb in range(S // 128):
                ps = psS.tile([128, WIN], F32, tag="psS")
                for half in range(WIN // SW):
                    s0 = w * WIN + half * SW
                    nc.tensor.matmul(
                        ps[:, half * SW : (half + 1) * SW],
                        kT_sb[p0 : p0 + 64, mi, tb * 128 : (tb + 1) * 128],
                        qT_sb[p0 : p0 + 64, mi, s0 : s0 + SW],
                        start=True,
                        stop=True,
                    )
                nc.scalar.activation(
                    out=pT_sb[:, tb, :], in_=ps[:], func=EXP,
                    scale=1.0 / float(np.sqrt(DH)),
                )
            pa = psAtt.tile([65, WIN], F32, tag="psAtt")
            for half in range(WIN // SW):
                for tb in range(S // 128):
                    nc.tensor.matmul(
                        pa[:, half * SW : (half + 1) * SW],
                        v_sb[:, tb, h, :],
                        pT_sb[:, tb, half * SW : (half + 1) * SW],
                        start=(tb == 0),
                        stop=(tb == S // 128 - 1),
                    )
            rec = small.tile([1, WIN], F32, tag="rec")
            nc.vector.reciprocal(rec[:], pa[64:65, :])
            bc = small.tile([64, WIN], F32, tag="bc")
            nc.gpsimd.partition_broadcast(bc[:], rec[:])
            nc.vector.tensor_mul(
                attT_sb[p0 : p0 + 64, mi, w * WIN : (w + 1) * WIN],
                pa[0:64, :],
                bc[:],
            )
        # out projection for this s-window (partial over local heads)
        for sm in range(WIN // 128):
            s0 = w * WIN + sm * 128
            for eb in range(E // SW):
                po = psS.tile([128, WIN], F32, tag="psS")
                for kb in range(2):
                    nc.tensor.matmul(
                        po[:, 0:SW],
                        attT_sb[:, kb, s0 : s0 + 128],
                        woT_sb[:, kb, eb * SW : (eb + 1) * SW],
                        start=(kb == 0),
                        stop=(kb == 1),
                    )
                ot = outsb.tile([128, SW], F32, tag="ot")
                nc.vector.tensor_copy(ot[:], po[:, 0:SW])
                nc.sync.dma_start(
                    out[s0 : s0 + 128, eb * SW : (eb + 1) * SW], ot[:]
                )


def _build_program(loop_n=None):
    """Build the per-core Bass program.

    loop_n: if set, wrap each phase body in a device-side For_i that
    repeats it loop_n times -- used only for timing (one NEFF dispatch
    amortizes the ~80ms axon RPC floor over loop_n kernel executions).
    """
    import contextlib

    import concourse.bacc as bacc
    import concourse.tile as tile
    from concourse import mybir

    F32 = mybir.dt.float32
    F32R = mybir.dt.float32r

    nc = bacc.Bacc("TRN2", target_bir_lowering=False)

    xT = nc.dram_tensor("xT", [E, S], F32R, kind="ExternalInput")
    wqT = nc.dram_tensor("wqT", [E, MLOC], F32R, kind="ExternalInput")
    wkT = nc.dram_tensor("wkT", [E, MLOC], F32R, kind="ExternalInput")
    wvT = nc.dram_tensor("wvT", [E, MLOC], F32R, kind="ExternalInput")
    woT = nc.dram_tensor("woT", [MLOC, E], F32R, kind="ExternalInput")
    bq = nc.dram_tensor("bq", [128, 2], F32, kind="ExternalInput")
    bk = nc.dram_tensor("bk", [128, 2], F32, kind="ExternalInput")
    bv = nc.dram_tensor("bv", [128, MLOC], F32, kind="ExternalInput")
    out = nc.dram_tensor("out", [S, E], F32, kind="ExternalOutput")

    def _loop(tc):
        if loop_n is None:
            return contextlib.nullcontext()
        return tc.For_i(
            0, loop_n, 1,
            hint_engines=(mybir.EngineType.PE, mybir.EngineType.Activation,
                          mybir.EngineType.DVE),
        )

    with tile.TileContext(nc) as tc:
        with (
            tc.tile_pool(name="consts", bufs=1) as consts,
            tc.tile_pool(name="qk", bufs=1) as qk,
            tc.tile_pool(name="vv", bufs=1) as vv,
            tc.tile_pool(name="att", bufs=1) as att,
        ):
            woT_sb = consts.tile([128, 2, E], F32R)
            nc.sync.dma_start(woT_sb[:], woT[:].rearrange("(c p) e -> p c e", p=128))
            bq_sb = consts.tile([128, 2], F32)
            bk_sb = consts.tile([128, 2], F32)
            bv_sb = consts.tile([128, MLOC], F32)
            nc.sync.dma_start(bq_sb[:], bq[:])
            nc.sync.dma_start(bk_sb[:], bk[:])
            nc.sync.dma_start(bv_sb[:], bv[:])
            ones1 = consts.tile([128, 1], F32)
            nc.vector.memset(ones1[:], 1.0)

            qT_sb = qk.tile([128, 2, S], F32R, tag="qT")
            kT_sb = qk.tile([128, 2, S], F32R, tag="kT")
            v_sb = vv.tile([128, S // 128, NH, DH + 1], F32R)
            attT_sb = att.tile([128, 2, S], F32R)

            with (
                tc.tile_pool(name="xw", bufs=1) as xw,
                tc.tile_pool(name="psA", bufs=4, space="PSUM") as psA,
            ):
                with _loop(tc):
                    _phase_a(nc, mybir, xw, psA, xT, wqT, wkT, wvT,
                             bq_sb, bk_sb, bv_sb, ones1,
                             qT_sb, kT_sb, v_sb, F32, F32R)

            with (
                tc.tile_pool(name="pp", bufs=1) as pp,
                tc.tile_pool(name="small", bufs=3) as small,
                tc.tile_pool(name="outsb", bufs=3) as outsb,
                tc.tile_pool(name="psS", bufs=3, space="PSUM") as psS,
                tc.tile_pool(name="psAtt", bufs=1, space="PSUM") as psAtt,
            ):
                with _loop(tc):
                    _phase_b(nc, mybir, pp, small, outsb, psS, psAtt,
                             qT_sb, kT_sb, v_sb, attT_sb, woT_sb, out,
                             F32, F32R)

    nc.compile()
    return nc


def _get_program():
    if "nc" not in _CACHE:
        _CACHE["nc"] = _build_program()
    return _CACHE["nc"]


def _make_in_maps(inputs):
    x = np.asarray(inputs["x"], dtype=np.float32)
    Wq = np.asarray(inputs["Wq"], dtype=np.float32)
    Wk = np.asarray(inputs["Wk"], dtype=np.float32)
    Wv = np.asarray(inputs["Wv"], dtype=np.float32)
    Wo = np.asarray(inputs["Wo"], dtype=np.float32)
    bq = np.asarray(inputs["bq"], dtype=np.float32)
    bk = np.asarray(inputs["bk"], dtype=np.float32)
    bv = np.asarray(inputs["bv"], dtype=np.float32)
    in_maps = []
    for c in range(8):
        b, g = divmod(c, 4)
        sl = slice(MLOC * g, MLOC * (g + 1))
        in_maps.append({
            "xT": np.ascontiguousarray(x[b].T),
            "wqT": np.ascontiguousarray(Wq[sl, :].T),
            "wkT": np.ascontiguousarray(Wk[sl, :].T),
            "wvT": np.ascontiguousarray(Wv[sl, :].T),
            "woT": np.ascontiguousarray(Wo[:, sl].T),
            "bq": np.ascontiguousarray(bq[sl].reshape(2, 128).T),
            "bk": np.ascontiguousarray(bk[sl].reshape(2, 128).T),
            "bv": np.ascontiguousarray(np.broadcast_to(bv[sl], (128, MLOC))),
        })
    return in_maps


def kernel(**inputs):
    from concourse.bass_utils import run_bass_kernel_spmd

    bo = np.asarray(inputs["bo"], dtype=np.float32)
    nc = _get_program()
    in_maps = _make_in_maps(inputs)
    res = run_bass_kernel_spmd(nc, in_maps, core_ids=list(range(8)))
    outs = [res.results[c]["out"] for c in range(8)]
    full = np.stack([
        outs[0] + outs[1] + outs[2] + outs[3] + bo,
        outs[4] + outs[5] + outs[6] + outs[7] + bo,
    ]).astype(np.float32)
    cw = np.ones((B, S, 1), dtype=np.float32)
    return full, cw


# revision 8
# speedup vs baseline: 1.0781x; 1.0781x over previous
"""Trainium2 Bass kernel for CohomologicalAttention.

The reference module is, mathematically, standard multi-head attention:
its "consistency weights" are a softmax over a size-1 axis and are
therefore identically 1.0 (and multiply the scores by 1.0), so the
restriction-map branch contributes nothing to either output.

Sharding (8 NeuronCores): data-parallel over batch (2) x tensor-parallel
over heads (4 heads per core).  Each core computes q/k/v projections for
its 4 heads, the full attention for those heads, and a partial output
projection (row-parallel in Wo); the host sums the 4 partials per batch.

All matmuls run as float32r (TF32-like fast fp32 path, 4x the fp32 rate)
accumulating in fp32 PSUM.  Softmax skips the max-subtraction (scores are
~N(0,1); |scores| < ~7 so exp is safe in fp32) and gets its denominator
for free from an appended ones-column on V.

Schedule: phase A iterates the contraction (e-chunk) outermost so the
first matmuls start as soon as the first xT chunk lands; phase B runs a
software pipeline over the 8 (window, head) slots -- scores+exp of slot
i interleaved with attend-V of slot i-1 at t-chunk granularity -- to keep
both the TensorE and the Activation engine (exp is ~147us of ACT work,
the pacer) continuously busy.  Out-projection chunks drain inside later
slots' streams.
"""

import numpy as np

B, S, E, H, P = 2, 2048, 1024, 16, 4
DH = E // H          # 64
NH = H // 4          # 4 local heads per core (8 cores = 2 batches x 4 groups)
MLOC = NH * DH       # 256 local q/k/v rows per core
KC = E // 128        # 8 contraction chunks of 128
TC = S // 128        # 16 t-chunks
SW = 512             # matmul moving-dim width (fp32 max)
WIN = 1024           # exp / s window
NWIN = S // WIN      # 2

_CACHE = {}


def _phase_a(nc, mybir, xw, psA, xT, wqT, wkT, wvT,
             bq_sb, bk_sb, bv_sb, ones1, qT_sb, kT_sb, v_sb, F32, F32R):
    wq_sb = xw.tile([128, KC, MLOC], F32R, tag="wq")
    wk_sb = xw.tile([128, KC, MLOC], F32R, tag="wk")
    wv_sb = xw.tile([128, KC, MLOC], F32R, tag="wv")
    nc.sync.dma_start(wq_sb[:], wqT[:].rearrange("(c p) m -> p c m", p=128))
    nc.sync.dma_start(wk_sb[:], wkT[:].rearrange("(c p) m -> p c m", p=128))
    nc.sync.dma_start(wv_sb[:], wvT[:].rearrange("(c p) m -> p c m", p=128))
    xT_sb = xw.tile([128, KC, S], F32R, tag="xT")
    for kb in range(KC):
        nc.sync.dma_start(xT_sb[:, kb, :], xT[kb * 128 : (kb + 1) * 128, :])

    # q/k passes: contraction (kb) outermost so matmuls chase the xT DMAs;
    # 8 live PSUM tiles per pass (= all 8 banks).
    for w_sb, b_sb, dst in ((wq_sb, bq_sb, qT_sb), (wk_sb, bk_sb, kT_sb)):
        tiles = [psA.tile([128, SW], F32, tag="psA", name=f"psA_{i}") for i in range(8)]
        for kb in range(KC):
            for mi in range(2):
                for nb in range(S // SW):
                    nc.tensor.matmul(
                        tiles[mi * 4 + nb][:],
                        w_sb[:, kb, mi * 128 : (mi + 1) * 128],
                        xT_sb[:, kb, nb * SW : (nb + 1) * SW],
                        start=(kb == 0),
                        stop=(kb == KC - 1),
                    )
        for mi in range(2):
            for nb in range(S // SW):
                nc.vector.tensor_scalar_add(
                    dst[:, mi, nb * SW : (nb + 1) * SW],
                    tiles[mi * 4 + nb][:],
                    b_sb[:, mi : mi + 1],
                )

    # v in natural [t, d] layout (+ ones column per head), two passes of 8
    # t-chunks to stay within the 8 PSUM banks.
    for half in range(2):
        tbs = range(half * 8, half * 8 + 8)
        tiles = {tb: psA.tile([128, MLOC], F32, tag="psA", name=f"psV_{tb}") for tb in tbs}
        for kb in range(KC):
            for tb in tbs:
                nc.tensor.matmul(
                    tiles[tb][:],
                    xT_sb[:, kb, tb * 128 : (tb + 1) * 128],
                    wv_sb[:, kb, :],
                    start=(kb == 0),
                    stop=(kb == KC - 1),
                )
        for tb in tbs:
            nc.vector.tensor_add(
                v_sb[:, tb, :, 0:DH],
                tiles[tb][:].rearrange("p (h d) -> p h d", h=NH),
                bv_sb[:].rearrange("p (h d) -> p h d", h=NH),
            )
    nc.vector.tensor_copy(
        v_sb[:, :, :, DH : DH + 1],
        ones1[:, 0:1, None].to_broadcast((128, TC, NH, 1)),
    )


def _phase_b(nc, mybir, pp, small, outsb, psS, psAtt,
             qT_sb, kT_sb, v_sb, attT_sb, woT_sb, out, F32, F32R):
    EXP = mybir.ActivationFunctionType.Exp
    scale = 1.0 / float(np.sqrt(DH))
    streams = [(w, h) for w in range(NWIN) for h in range(NH)]
    outproj_queue = []

    def emit_attnv(prev, pa_prev, tb):
        w_p, h_p, pts_p = prev
        for half in range(WIN // SW):
            nc.tensor.matmul(
                pa_prev[:, half * SW : (half + 1) * SW],
                v_sb[:, tb, h_p, :],
                pts_p[tb][:, half * SW : (half + 1) * SW],
                start=(tb == 0),
                stop=(tb == TC - 1),
            )

    def emit_norm(prev, pa_prev):
        w_p, h_p, _ = prev
        p0 = (h_p % 2) * 64
        mi = h_p // 2
        rec = small.tile([1, WIN], F32, tag="rec")
        nc.vector.reciprocal(rec[:], pa_prev[64:65, :])
        bc = small.tile([64, WIN], F32, tag="bc")
        nc.gpsimd.partition_broadcast(bc[:], rec[:])
        nc.vector.tensor_mul(
            attT_sb[p0 : p0 + 64, mi, w_p * WIN : (w_p + 1) * WIN],
            pa_prev[0:64, :],
            bc[:],
        )
        if h_p == NH - 1:
            for sm in range(WIN // 128):
                for eb in range(E // SW):
                    outproj_queue.append((w_p * WIN + sm * 128, eb))

    def emit_outproj_chunk():
        s0, eb = outproj_queue.pop(0)
        po = psS.tile([128, WIN], F32, tag="sc")
        for kb in range(2):
            nc.tensor.matmul(
                po[:, 0:SW],
                attT_sb[:, kb, s0 : s0 + 128],
                woT_sb[:, kb, eb * SW : (eb + 1) * SW],
                start=(kb == 0),
                stop=(kb == 1),
            )
        ot = outsb.tile([128, SW], F32, tag="ot")
        nc.vector.tensor_copy(ot[:], po[:, 0:SW])
        nc.sync.dma_start(out[s0 : s0 + 128, eb * SW : (eb + 1) * SW], ot[:])

    prev = None
    pa_prev = None
    for w, h in streams:
        p0 = (h % 2) * 64
        mi = h // 2
        pts = []
        pa_cur = (psAtt.tile([65, WIN], F32, tag="att", name="pa") if prev is not None else None)
        for tb in range(TC):
            ps = psS.tile([128, WIN], F32, tag="sc")
            for half in range(WIN // SW):
                s0 = w * WIN + half * SW
                nc.tensor.matmul(
                    ps[:, half * SW : (half + 1) * SW],
                    kT_sb[p0 : p0 + 64, mi, tb * 128 : (tb + 1) * 128],
                    qT_sb[p0 : p0 + 64, mi, s0 : s0 + SW],
                    start=True,
                    stop=True,
                )
            pt = pp.tile([128, WIN], F32R, tag="pT", name="pt")
            nc.scalar.activation(out=pt[:], in_=ps[:], func=EXP, scale=scale)
            pts.append(pt)
            if prev is not None:
                emit_attnv(prev, pa_cur, tb)
            if outproj_queue:
                emit_outproj_chunk()
        if prev is not None:
            emit_norm(prev, pa_cur)
        prev = (w, h, pts)

    # tail: attend-V + norm for the last slot, then remaining out-proj chunks
    pa_cur = psAtt.tile([65, WIN], F32, tag="att", name="pa")
    for tb in range(TC):
        emit_attnv(prev, pa_cur, tb)
        if outproj_queue:
            emit_outproj_chunk()
    emit_norm(prev, pa_cur)
    while outproj_queue:
        emit_outproj_chunk()


def _build_program(loop_n=None):
    """Build the per-core Bass program.

    loop_n: if set, wrap each phase body in a device-side For_i that
    repeats it loop_n times -- used only for timing (one NEFF dispatch
    amortizes the ~80ms axon RPC floor over loop_n kernel executions).
    """
    import contextlib

    import concourse.bacc as bacc
    import concourse.tile as tile
    from concourse import mybir

    F32 = mybir.dt.float32
    F32R = mybir.dt.float32r

    nc = bacc.Bacc("TRN2", target_bir_lowering=False)

    xT = nc.dram_tensor("xT", [E, S], F32R, kind="ExternalInput")
    wqT = nc.dram_tensor("wqT", [E, MLOC], F32R, kind="ExternalInput")
    wkT = nc.dram_tensor("wkT", [E, MLOC], F32R, kind="ExternalInput")
    wvT = nc.dram_tensor("wvT", [E, MLOC], F32R, kind="ExternalInput")
    woT = nc.dram_tensor("woT", [MLOC, E], F32R, kind="ExternalInput")
    bq = nc.dram_tensor("bq", [128, 2], F32, kind="ExternalInput")
    bk = nc.dram_tensor("bk", [128, 2], F32, kind="ExternalInput")
    bv = nc.dram_tensor("bv", [128, MLOC], F32, kind="ExternalInput")
    out = nc.dram_tensor("out", [S, E], F32, kind="ExternalOutput")

    def _loop(tc):
        if loop_n is None:
            return contextlib.nullcontext()
        return tc.For_i(
            0, loop_n, 1,
            hint_engines=(mybir.EngineType.PE, mybir.EngineType.Activation,
                          mybir.EngineType.DVE),
        )

    with tile.TileContext(nc) as tc:
        with (
            tc.tile_pool(name="consts", bufs=1) as consts,
            tc.tile_pool(name="qk", bufs=1) as qk,
            tc.tile_pool(name="vv", bufs=1) as vv,
            tc.tile_pool(name="att", bufs=1) as att,
        ):
            woT_sb = consts.tile([128, 2, E], F32R)
            nc.sync.dma_start(woT_sb[:], woT[:].rearrange("(c p) e -> p c e", p=128))
            bq_sb = consts.tile([128, 2], F32)
            bk_sb = consts.tile([128, 2], F32)
            bv_sb = consts.tile([128, MLOC], F32)
            nc.sync.dma_start(bq_sb[:], bq[:])
            nc.sync.dma_start(bk_sb[:], bk[:])
            nc.sync.dma_start(bv_sb[:], bv[:])
            ones1 = consts.tile([128, 1], F32)
            nc.vector.memset(ones1[:], 1.0)

            qT_sb = qk.tile([128, 2, S], F32R, tag="qT")
            kT_sb = qk.tile([128, 2, S], F32R, tag="kT")
            v_sb = vv.tile([128, TC, NH, DH + 1], F32R)
            attT_sb = att.tile([128, 2, S], F32R)

            with (
                tc.tile_pool(name="xw", bufs=1) as xw,
                tc.tile_pool(name="psA", bufs=8, space="PSUM") as psA,
            ):
                with _loop(tc):
                    _phase_a(nc, mybir, xw, psA, xT, wqT, wkT, wvT,
                             bq_sb, bk_sb, bv_sb, ones1,
                             qT_sb, kT_sb, v_sb, F32, F32R)

            with (
                tc.tile_pool(name="pp", bufs=18) as pp,
                tc.tile_pool(name="small", bufs=3) as small,
                tc.tile_pool(name="outsb", bufs=3) as outsb,
                tc.tile_pool(name="psS", bufs=2, space="PSUM") as psS,
                tc.tile_pool(name="psAtt", bufs=2, space="PSUM") as psAtt,
            ):
                with _loop(tc):
                    _phase_b(nc, mybir, pp, small, outsb, psS, psAtt,
                             qT_sb, kT_sb, v_sb, attT_sb, woT_sb, out,
                             F32, F32R)

    nc.compile()
    return nc


def _get_program():
    if "nc" not in _CACHE:
        _CACHE["nc"] = _build_program()
    return _CACHE["nc"]


def _make_in_maps(inputs):
    x = np.asarray(inputs["x"], dtype=np.float32)
    Wq = np.asarray(inputs["Wq"], dtype=np.float32)
    Wk = np.asarray(inputs["Wk"], dtype=np.float32)
    Wv = np.asarray(inputs["Wv"], dtype=np.float32)
    Wo = np.asarray(inputs["Wo"], dtype=np.float32)
    bq = np.asarray(inputs["bq"], dtype=np.float32)
    bk = np.asarray(inputs["bk"], dtype=np.float32)
    bv = np.asarray(inputs["bv"], dtype=np.float32)
    in_maps = []
    for c in range(8):
        b, g = divmod(c, 4)
        sl = slice(MLOC * g, MLOC * (g + 1))
        in_maps.append({
            "xT": np.ascontiguousarray(x[b].T),
            "wqT": np.ascontiguousarray(Wq[sl, :].T),
            "wkT": np.ascontiguousarray(Wk[sl, :].T),
            "wvT": np.ascontiguousarray(Wv[sl, :].T),
            "woT": np.ascontiguousarray(Wo[:, sl].T),
            "bq": np.ascontiguousarray(bq[sl].reshape(2, 128).T),
            "bk": np.ascontiguousarray(bk[sl].reshape(2, 128).T),
            "bv": np.ascontiguousarray(np.broadcast_to(bv[sl], (128, MLOC))),
        })
    return in_maps


def kernel(**inputs):
    from concourse.bass_utils import run_bass_kernel_spmd

    bo = np.asarray(inputs["bo"], dtype=np.float32)
    nc = _get_program()
    in_maps = _make_in_maps(inputs)
    res = run_bass_kernel_spmd(nc, in_maps, core_ids=list(range(8)))
    outs = [res.results[c]["out"] for c in range(8)]
    full = np.stack([
        outs[0] + outs[1] + outs[2] + outs[3] + bo,
        outs[4] + outs[5] + outs[6] + outs[7] + bo,
    ]).astype(np.float32)
    cw = np.ones((B, S, 1), dtype=np.float32)
    return full, cw
